# revision 17
# baseline (speedup 1.0000x reference)
"""Per-pixel affine transform (bilateral-grid style) on 8 TRN2 NeuronCores.

Reference computation (per batch b, pixel (h, w)):
    out[d] = sum_{c=0..2} x[c] * A[c, d] + A[3, d]
where A[c_in, d] = coeff channel c_in*3 + d.

Sharding: pure data parallel over batch B=8 -> 1 batch per core.
Per-core layout: pixels flattened to [128 partitions, 8192 free]; the
free dim is processed in chunks; each DRAM stream is pre-blocked on the
host so one chunk of one stream is a single contiguous [3, w] block per
partition row -> every DMA is 128 fat descriptors (2.3-6 KiB), which
keeps the HWDGE descriptor-generation rate and the DMA bus efficient.

HBM traffic is the roofline, so the three mult groups of coeff (9 of 18
streamed channels) are int8-quantized on the host with per-(batch, group)
symmetric scales s_g = max|A_g|/127, folded into x on the host
(x'_c = s_c * x_c, fp16) so on-device dequant is a plain int8->fp16
upcast. The bias group stays fp16: its add would otherwise need a GPSIMD
pass, and GPSIMD elementwise ops (Q7 software) stall concurrent DVE
instructions -- measured 106us -> 135us when the final add ran on
GPSIMD. Per-core traffic: 9 MiB coeff(int8) + 6 MiB bias(fp16) +
6 MiB x(fp16) + 6 MiB out = 27 MiB vs 36 MiB all-fp16.
Max rel-err ~6.5e-3 (gate 2e-2).

Engine split (per chunk of width w, all passes 3w elems/partition):
  Act : up_c = upcast(q_c), c=0..2       (int8 operands would force DVE
                                          to 1x mode; Act copies at
                                          0.83 ns/elem are cheaper)
  DVE : m_c = up_c * x'_c (broadcast over d), a = m0+m1+m2+bias
        (6 passes, all-fp16 SBUF -> 2x mode)
  Pool: SWDGE store dispatch only (no compute -- see above)

DMA rings: sync (all loads; SP is otherwise idle so dispatch never queues
behind compute), gpsimd SWDGE stores; last store on scalar HWDGE for
drain latency.
"""

import os
import sys

for _p in ("/opt/trn_rl_repo",):
    if _p not in sys.path and os.path.isdir(_p):
        sys.path.append(_p)

import numpy as np

import concourse.bacc as bacc
import concourse.mybir as mybir
from concourse.bass_utils import run_bass_kernel_spmd
from concourse.tile import TileContext

B = 8
P = 128          # SBUF partitions
FREE = 8192      # pixels per partition (1024*1024 / 128)
F = 1024         # max free-dim chunk
N_CORES = 8

# small first/last chunks for pipeline ramp/drain; 3*w*dtype >= 512 B
# everywhere so no descriptor-rate penalty
WIDTHS = [128, 256, 512] + [1024] * 6 + [768, 256, 128]
assert sum(WIDTHS) == FREE

_cached_nc = None


def _build_nc():
    nc = bacc.Bacc("TRN2", target_bir_lowering=False, debug=False)
    f16 = mybir.dt.float16
    i8 = mybir.dt.int8

    # chunk-blocked streams: row p of chunk ci holds [3, w_ci] contiguous
    q = nc.dram_tensor("q", [3, P, 3 * FREE], i8, kind="ExternalInput").ap()
    bias = nc.dram_tensor("bias", [P, 3 * FREE], f16, kind="ExternalInput").ap()
    x = nc.dram_tensor("x", [P, 3 * FREE], f16, kind="ExternalInput").ap()
    out = nc.dram_tensor("out", [P, 3 * FREE], f16, kind="ExternalOutput").ap()

    with TileContext(nc) as tc:
        with tc.tile_pool(name="p", bufs=3) as pool:
            o0 = 0
            for ci, w in enumerate(WIDTHS):
                os_ = slice(o0, o0 + 3 * w)
                o0 += 3 * w
                last = ci == len(WIDTHS) - 1

                # all loads on the sync ring: q0 first (Act's first
                # dependency), then x (DVE's), bias last (needed last).
                # Extra buffers on the tiles whose recycling is gated by
                # late readers (u0 by the store, b by the final add).
                Q = [
                    pool.tile([P, 3 * F], i8, tag=f"q{g}", name=f"q{g}")
                    for g in range(3)
                ]
                X = pool.tile([P, 3 * F], f16, tag="x", bufs=4)
                Bt = pool.tile([P, 3 * F], f16, tag="b", bufs=4)

                nc.sync.dma_start(out=Q[0][:, : 3 * w], in_=q[0, :, os_])
                nc.sync.dma_start(out=X[:, : 3 * w], in_=x[:, os_])
                nc.sync.dma_start(out=Q[1][:, : 3 * w], in_=q[1, :, os_])
                nc.sync.dma_start(out=Q[2][:, : 3 * w], in_=q[2, :, os_])
                nc.sync.dma_start(out=Bt[:, : 3 * w], in_=bias[:, os_])

                # Act: plain upcast int8 -> fp16 (scales already folded
                # into x), interleaved per group so DVE can start on up0
                # while Act runs up1/up2
                U = []
                for c in range(3):
                    Uc = pool.tile(
                        [P, 3 * F], f16, tag=f"u{c}", name=f"u{c}",
                        bufs=4 if c == 0 else 3,
                    )
                    U.append(Uc)
                    nc.scalar.copy(out=Uc[:, : 3 * w], in_=Q[c][:, : 3 * w])

                # DVE: all-fp16 2x-mode passes; m_c in place over up_c,
                # accumulate into U[0]. a1 fills the wait for Act's third
                # upcast; bias add last.
                def mult(c):
                    Ucv = U[c][:, : 3 * w].rearrange("p (d f) -> p d f", d=3)
                    xc3 = (
                        X[:, c * w : (c + 1) * w]
                        .unsqueeze(1)
                        .broadcast_to([P, 3, w])
                    )
                    nc.vector.tensor_tensor(Ucv, Ucv, xc3, mybir.AluOpType.mult)

                def add(src):
                    nc.vector.tensor_add(
                        U[0][:, : 3 * w], U[0][:, : 3 * w], src[:, : 3 * w]
                    )

                mult(0)
                mult(1)
                add(U[1])
                mult(2)
                add(U[2])
                add(Bt)

                # final store rides the (by-then idle) scalar HWDGE ring:
                # lower latency than SWDGE on the drain-critical path
                store_eng = nc.scalar if last else nc.gpsimd
                store_eng.dma_start(out=out[:, os_], in_=U[0][:, : 3 * w])
    nc.compile()
    return nc


def _get_nc():
    global _cached_nc
    if _cached_nc is None:
        _cached_nc = _build_nc()
    return _cached_nc


def _interleave(a3):
    """[3, P, FREE] -> [P, 3*FREE] with per-chunk [3, w] blocks per row."""
    blocks, j0 = [], 0
    for w in WIDTHS:
        blocks.append(
            np.ascontiguousarray(a3[:, :, j0 : j0 + w].transpose(1, 0, 2)).reshape(
                P, 3 * w
            )
        )
        j0 += w
    return np.concatenate(blocks, axis=1)


def _deinterleave(flat):
    """[P, 3*FREE] with per-chunk [3, w] blocks -> [3, P, FREE]."""
    outs, o = [], 0
    for w in WIDTHS:
        outs.append(flat[:, o : o + 3 * w].reshape(P, 3, w).transpose(1, 0, 2))
        o += 3 * w
    return np.concatenate(outs, axis=2)


def _make_in_maps(coeff, x):
    """coeff [B,12,1024,1024] f32, x [B,3,1024,1024] f32 -> per-core maps."""
    A = np.ascontiguousarray(coeff, dtype=np.float32).reshape(B, 4, 3, 1024, 1024)
    s = (np.abs(A[:, :3]).max(axis=(2, 3, 4)) / 127.0).astype(np.float32)  # [B,3]
    maps = []
    for i in range(B):
        qi = np.clip(
            np.rint(A[i, :3] * (1.0 / s[i])[:, None, None, None]), -127, 127
        ).astype(np.int8)
        xi = (s[i][:, None, None] * np.asarray(x[i], dtype=np.float32)).astype(
            np.float16
        )
        maps.append(
            {
                "q": np.stack(
                    [_interleave(qi[g].reshape(3, P, FREE)) for g in range(3)]
                ),
                "bias": _interleave(
                    A[i, 3].astype(np.float16).reshape(3, P, FREE)
                ),
                "x": _interleave(xi.reshape(3, P, FREE)),
            }
        )
    return maps


def kernel(coeff, full_res_input):
    assert coeff.shape == (B, 12, 1024, 1024) and full_res_input.shape == (
        B,
        3,
        1024,
        1024,
    )
    nc = _get_nc()
    in_maps = _make_in_maps(coeff, full_res_input)
    res = run_bass_kernel_spmd(nc, in_maps, list(range(N_CORES))).results
    return np.stack(
        [
            _deinterleave(res[i]["out"]).reshape(3, 1024, 1024)
            for i in range(B)
        ]
    ).astype(np.float32)


# revision 20
# speedup vs baseline: 1.0245x; 1.0245x over previous
"""Per-pixel affine transform (bilateral-grid style) on 8 TRN2 NeuronCores.

Reference computation (per batch b, pixel (h, w)):
    out[d] = sum_{c=0..2} x[c] * A[c, d] + A[3, d]
where A[c_in, d] = coeff channel c_in*3 + d.

Sharding: pure data parallel over batch B=8 -> 1 batch per core.
Per-core layout: pixels flattened to [128 partitions, 8192 free]; the
free dim is processed in chunks; each DRAM stream is pre-blocked on the
host so one chunk of one stream is a single contiguous [3, w] block per
partition row -> every DMA is 128 fat descriptors (2.3-6 KiB), which
keeps the HWDGE descriptor-generation rate and the DMA bus efficient.

HBM traffic is the roofline, so the three mult groups of coeff (9 of 18
streamed channels) are int8-quantized on the host with per-(batch, group)
symmetric scales s_g = max|A_g|/127, folded into x on the host
(x'_c = s_c * x_c, fp16) so on-device dequant is a plain int8->fp16
upcast. The bias group stays fp16: its add would otherwise need a GPSIMD
pass, and GPSIMD elementwise ops (Q7 software) stall concurrent DVE
instructions -- measured 106us -> 135us when the final add ran on
GPSIMD. Per-core traffic: 9 MiB coeff(int8) + 6 MiB bias(fp16) +
6 MiB x(fp16) + 6 MiB out = 27 MiB vs 36 MiB all-fp16.
Max rel-err ~6.5e-3 (gate 2e-2).

Engine split (per chunk of width w, all passes 3w elems/partition):
  Act : up_c = upcast(q_c), c=0..2       (int8 operands would force DVE
                                          to 1x mode; Act copies at
                                          0.83 ns/elem are cheaper)
  DVE : m_c = up_c * x'_c (broadcast over d), a = m0+m1+m2+bias
        (6 passes, all-fp16 SBUF -> 2x mode)
  Pool: SWDGE store dispatch only (no compute -- see above)

DMA rings: sync (all loads; SP is otherwise idle so dispatch never queues
behind compute), gpsimd SWDGE stores; last store on scalar HWDGE for
drain latency.
"""

import os
import sys

for _p in ("/opt/trn_rl_repo",):
    if _p not in sys.path and os.path.isdir(_p):
        sys.path.append(_p)

import numpy as np

import concourse.bacc as bacc
import concourse.mybir as mybir
from concourse.bass_utils import run_bass_kernel_spmd
from concourse.tile import TileContext

B = 8
P = 128          # SBUF partitions
FREE = 8192      # pixels per partition (1024*1024 / 128)
F = 1024         # max free-dim chunk
N_CORES = 8

# small first/last chunks for pipeline ramp/drain; 3*w*dtype >= 512 B
# everywhere so no descriptor-rate penalty
WIDTHS = [256, 512, 768] + [1024] * 5 + [768, 512, 256]
assert sum(WIDTHS) == FREE

_cached_nc = None


def _build_nc():
    nc = bacc.Bacc("TRN2", target_bir_lowering=False, debug=False)
    f16 = mybir.dt.float16
    i8 = mybir.dt.int8

    # chunk-blocked streams: row p of chunk ci holds [3, w_ci] contiguous
    q = nc.dram_tensor("q", [3, P, 3 * FREE], i8, kind="ExternalInput").ap()
    bias = nc.dram_tensor("bias", [P, 3 * FREE], f16, kind="ExternalInput").ap()
    x = nc.dram_tensor("x", [P, 3 * FREE], f16, kind="ExternalInput").ap()
    out = nc.dram_tensor("out", [P, 3 * FREE], f16, kind="ExternalOutput").ap()

    with TileContext(nc) as tc:
        with tc.tile_pool(name="p", bufs=3) as pool:
            o0 = 0
            for ci, w in enumerate(WIDTHS):
                os_ = slice(o0, o0 + 3 * w)
                o0 += 3 * w
                last = ci == len(WIDTHS) - 1

                # all loads on the sync ring: q0 first (Act's first
                # dependency), then x (DVE's), bias last (needed last).
                # Extra buffers on the tiles whose recycling is gated by
                # late readers (u0 by the store, b by the final add).
                Q = [
                    pool.tile([P, 3 * F], i8, tag=f"q{g}", name=f"q{g}", bufs=4)
                    for g in range(3)
                ]
                X = pool.tile([P, 3 * F], f16, tag="x", bufs=4)
                Bt = pool.tile([P, 3 * F], f16, tag="b", bufs=4)

                nc.sync.dma_start(out=Q[0][:, : 3 * w], in_=q[0, :, os_])
                nc.sync.dma_start(out=X[:, : 3 * w], in_=x[:, os_])
                nc.sync.dma_start(out=Q[1][:, : 3 * w], in_=q[1, :, os_])
                nc.sync.dma_start(out=Q[2][:, : 3 * w], in_=q[2, :, os_])
                nc.sync.dma_start(out=Bt[:, : 3 * w], in_=bias[:, os_])

                # Act: plain upcast int8 -> fp16 (scales already folded
                # into x), interleaved per group so DVE can start on up0
                # while Act runs up1/up2
                U = []
                for c in range(3):
                    Uc = pool.tile(
                        [P, 3 * F], f16, tag=f"u{c}", name=f"u{c}", bufs=4
                    )
                    U.append(Uc)
                    nc.scalar.copy(out=Uc[:, : 3 * w], in_=Q[c][:, : 3 * w])

                # DVE: all-fp16 2x-mode passes; m_c in place over up_c,
                # accumulate into U[0]. a1 fills the wait for Act's third
                # upcast; bias add last.
                def mult(c):
                    Ucv = U[c][:, : 3 * w].rearrange("p (d f) -> p d f", d=3)
                    xc3 = (
                        X[:, c * w : (c + 1) * w]
                        .unsqueeze(1)
                        .broadcast_to([P, 3, w])
                    )
                    nc.vector.tensor_tensor(Ucv, Ucv, xc3, mybir.AluOpType.mult)

                def add(src):
                    nc.vector.tensor_add(
                        U[0][:, : 3 * w], U[0][:, : 3 * w], src[:, : 3 * w]
                    )

                mult(0)
                mult(1)
                add(U[1])
                mult(2)
                add(U[2])
                add(Bt)

                # final store rides the (by-then idle) scalar HWDGE ring:
                # lower latency than SWDGE on the drain-critical path
                store_eng = nc.scalar if last else nc.gpsimd
                store_eng.dma_start(out=out[:, os_], in_=U[0][:, : 3 * w])
    nc.compile()
    return nc


def _get_nc():
    global _cached_nc
    if _cached_nc is None:
        _cached_nc = _build_nc()
    return _cached_nc


def _interleave(a3):
    """[3, P, FREE] -> [P, 3*FREE] with per-chunk [3, w] blocks per row."""
    blocks, j0 = [], 0
    for w in WIDTHS:
        blocks.append(
            np.ascontiguousarray(a3[:, :, j0 : j0 + w].transpose(1, 0, 2)).reshape(
                P, 3 * w
            )
        )
        j0 += w
    return np.concatenate(blocks, axis=1)


def _deinterleave(flat):
    """[P, 3*FREE] with per-chunk [3, w] blocks -> [3, P, FREE]."""
    outs, o = [], 0
    for w in WIDTHS:
        outs.append(flat[:, o : o + 3 * w].reshape(P, 3, w).transpose(1, 0, 2))
        o += 3 * w
    return np.concatenate(outs, axis=2)


def _make_in_maps(coeff, x):
    """coeff [B,12,1024,1024] f32, x [B,3,1024,1024] f32 -> per-core maps."""
    A = np.ascontiguousarray(coeff, dtype=np.float32).reshape(B, 4, 3, 1024, 1024)
    s = (np.abs(A[:, :3]).max(axis=(2, 3, 4)) / 127.0).astype(np.float32)  # [B,3]
    maps = []
    for i in range(B):
        qi = np.clip(
            np.rint(A[i, :3] * (1.0 / s[i])[:, None, None, None]), -127, 127
        ).astype(np.int8)
        xi = (s[i][:, None, None] * np.asarray(x[i], dtype=np.float32)).astype(
            np.float16
        )
        maps.append(
            {
                "q": np.stack(
                    [_interleave(qi[g].reshape(3, P, FREE)) for g in range(3)]
                ),
                "bias": _interleave(
                    A[i, 3].astype(np.float16).reshape(3, P, FREE)
                ),
                "x": _interleave(xi.reshape(3, P, FREE)),
            }
        )
    return maps


def kernel(coeff, full_res_input):
    assert coeff.shape == (B, 12, 1024, 1024) and full_res_input.shape == (
        B,
        3,
        1024,
        1024,
    )
    nc = _get_nc()
    in_maps = _make_in_maps(coeff, full_res_input)
    res = run_bass_kernel_spmd(nc, in_maps, list(range(N_CORES))).results
    return np.stack(
        [
            _deinterleave(res[i]["out"]).reshape(3, 1024, 1024)
            for i in range(B)
        ]
    ).astype(np.float32)
